# revision 1
# baseline (speedup 1.0000x reference)
"""Trainium2 Bass kernel for nn_GaussianSplatter (v2).

Data-parallel over batch: 2 images per core x 8 cores. Host does only
layout/dtype prep (transpose + fp8/bf16 casts) and the tiny [49,100]->W
Gaussian math between launches; all bulk reduction/matmul work is on-device.

launch 1 (reduce): logits shard as fp8 tiles [512 tiles x (100k x 49p)].
  PE sums 480 tiles via ones-LHS DoubleRow matmuls (2 supergroups x 10
  column-chunk matmuls, fp8 2x mode, PSUM f32 accumulate); DVE reduces 24
  tiles in one TensorReduce; Pool folds the last 8 via an add tree. Three
  f32 partials [100,49] DMA back; host combines across cores -> lbar -> W.

launch 2 (apply): feat pre-packed on host to patch-major bf16 [98, 2048]
  (98 = 2 images-halves x 49 patch positions). One 98x98 blockdiag(W,W)
  bf16 stationary matrix; 4 matmuls of 512 columns -> PSUM f32; DVE/Act
  copy-convert to bf16; DMA out. Host unpacks to [16,8,112,112] f32.
"""
import numpy as np
import ml_dtypes

import concourse.bass as bass
import concourse.mybir as mybir
from concourse.bass_utils import run_bass_kernel_spmd

N_CORES = 8
B, C_IN, H, W_ = 16, 64, 112, 112
K = 100
FEAT_C = 8
ROW = COL = 7
KSIZE = 5
P = 49
BPC = B // N_CORES           # images per core
NT = BPC * 16 * 16           # 512 tiles (b, nh, nw) per core
PLANE = 120                  # tiles per PE plane; 4 planes = 480 on PE
NT_PE = 4 * PLANE
NT_DVE = 24
NT_POOL = NT - NT_PE - NT_DVE   # 8
COLS = K * P                 # 4900
NDR = 8                      # DoubleRow column chunks (base-0 psum accumulators)
DRW = 512                    # cols per DR chunk
PLW = (COLS - NDR * DRW) // 2   # 402: leftover cols, 2 plain chunks at base 64
L2N = BPC * FEAT_C * 256 // 2   # 2048 columns per half in launch 2

FP8 = ml_dtypes.float8_e4m3fn
BF16 = ml_dtypes.bfloat16

_cache = {}


def _translate_bilinear_np(img, sx, sy):
    Pn, Hh, Ww = img.shape
    ii = np.arange(Hh, dtype=img.dtype)[None, :, None] + sy[:, None, None]
    jj = np.arange(Ww, dtype=img.dtype)[None, None, :] + sx[:, None, None]
    ii = np.broadcast_to(ii, (Pn, Hh, Ww))
    jj = np.broadcast_to(jj, (Pn, Hh, Ww))
    i0 = np.floor(ii)
    j0 = np.floor(jj)
    wi = ii - i0
    wj = jj - j0
    pidx = np.arange(Pn)[:, None, None]

    def gather(iz, jz):
        valid = (iz >= 0) & (iz <= Hh - 1) & (jz >= 0) & (jz <= Ww - 1)
        v = img[pidx, np.clip(iz, 0, Hh - 1).astype(np.int32),
                np.clip(jz, 0, Ww - 1).astype(np.int32)]
        return np.where(valid, v, np.zeros((), img.dtype))

    v00 = gather(i0, j0)
    v01 = gather(i0, j0 + 1.0)
    v10 = gather(i0 + 1.0, j0)
    v11 = gather(i0 + 1.0, j0 + 1.0)
    return v00 * (1 - wi) * (1 - wj) + v01 * (1 - wi) * wj \
        + v10 * wi * (1 - wj) + v11 * wi * wj


def _compute_W(lbar, sigma_x, sigma_y, opacity, rho):
    lbar = lbar.astype(np.float64)
    wsx = lbar @ sigma_x.astype(np.float64)
    wsy = lbar @ sigma_y.astype(np.float64)
    wop = lbar @ opacity[:, 0].astype(np.float64)
    wrho = lbar @ rho[:, 0].astype(np.float64)
    a = wsx ** 2 + 1e-5
    d = wsy ** 2 + 1e-5
    b = wrho * wsx * wsy
    det = a * d - b * b
    ia, ib, idd = d / det, -b / det, a / det
    ax = np.linspace(-5.0, 5.0, KSIZE)
    xx = ax[:, None]
    yy = ax[None, :]
    z = -0.5 * (ia[:, None, None] * xx ** 2 + 2.0 * ib[:, None, None] * xx * yy
                + idd[:, None, None] * yy ** 2)
    kern = np.exp(z) / (2.0 * np.pi * np.sqrt(det)[:, None, None])
    kern = kern / kern.max(axis=(-2, -1), keepdims=True)
    ph, pw = ROW - KSIZE, COL - KSIZE
    kern = np.pad(kern, ((0, 0), (ph // 2, ph - ph // 2), (pw // 2, pw - pw // 2)))
    rr, cc = np.meshgrid(np.arange(ROW, dtype=np.float64),
                         np.arange(COL, dtype=np.float64), indexing='ij')
    tx = 1.0 - 2.0 * cc.reshape(-1) / ROW
    ty = 1.0 - 2.0 * rr.reshape(-1) / COL
    kT = _translate_bilinear_np(kern, tx * (COL - 1) / 2.0, ty * (ROW - 1) / 2.0)
    return (wop[:, None] * kT.reshape(P, P)).astype(np.float32)


def _build_reduce_nc():
    nc = bass.Bass()
    f8 = mybir.dt.float8e4
    bf = mybir.dt.bfloat16
    f32 = mybir.dt.float32

    xpe = nc.declare_dram_parameter("xpe", [4, PLANE, COLS], f8, isOutput=False)
    xdv = nc.declare_dram_parameter("xdv", [K, NT_DVE, P], f8, isOutput=False)
    xpl = nc.declare_dram_parameter("xpl", [K, NT_POOL, P], f8, isOutput=False)
    onesd = nc.declare_dram_parameter("onesd", [PLANE, 128], f8, isOutput=False)
    pe_part = nc.declare_dram_parameter("pe_part", [NDR, DRW], f32, isOutput=True)
    pe_part2 = nc.declare_dram_parameter("pe_part2", [2, PLW], f32, isOutput=True)
    dvpl_part = nc.declare_dram_parameter("dvpl_part", [K, 2, P], f32, isOutput=True)

    with bass.ExitStack() as ctx:
        XPE = ctx.enter_context(nc.sbuf_tensor([PLANE, 4, COLS], f8))
        XDV = ctx.enter_context(nc.sbuf_tensor([K, NT_DVE, P], f8))
        XPL = ctx.enter_context(nc.sbuf_tensor([K, NT_POOL, P], f8))
        ONES = ctx.enter_context(nc.sbuf_tensor([PLANE, 2, 64], f8))
        PL4 = ctx.enter_context(nc.sbuf_tensor([K, 4, P], bf))
        DPP = ctx.enter_context(nc.sbuf_tensor([K, 2, P], f32))
        PEPS = ctx.enter_context(nc.sbuf_tensor([128, NDR, DRW], f32))
        pss = [ctx.enter_context(nc.psum_tensor(f"rps{b}", [128, DRW], f32))
               for b in range(NDR)]
        dpe = ctx.enter_context(nc.semaphore("dpe"))
        ddv = ctx.enter_context(nc.semaphore("ddv"))
        dpl = ctx.enter_context(nc.semaphore("dpl"))
        don = ctx.enter_context(nc.semaphore("don"))
        msem = ctx.enter_context(nc.semaphore("m"))
        csem = ctx.enter_context(nc.semaphore("c"))
        vsem = ctx.enter_context(nc.semaphore("v"))
        psem = ctx.enter_context(nc.semaphore("p"))
        osem = ctx.enter_context(nc.semaphore("o"))
        block = ctx.enter_context(nc.Block())

        @block.sync
        def _(sync):
            for pl in range(4):
                sync.dma_start(XPE[:, pl, :], xpe[pl]).then_inc(dpe, 16)
            sync.wait_ge(psem, 1)
            sync.wait_ge(vsem, 1)
            sync.dma_start(dvpl_part[:], DPP[:]).then_inc(osem, 16)
            sync.wait_ge(csem, NDR)
            sync.dma_start(pe_part[:], PEPS[0:1, :, :]).then_inc(osem, 16)
            sync.dma_start(pe_part2[:], PEPS[64:65, 0:2, 0:PLW]).then_inc(osem, 16)
            sync.wait_ge(osem, 48)

        @block.scalar
        def _(scalar):
            scalar.dma_start(ONES[:].rearrange("t a b -> t (a b)"),
                             onesd[:]).then_inc(don, 16)
            scalar.dma_start(XPL[:], xpl[:]).then_inc(dpl, 16)
            scalar.dma_start(XDV[:], xdv[:]).then_inc(ddv, 16)
            for b in (3, 5, 7):          # odd banks (DR rows only)
                scalar.wait_ge(msem, b + 1)
                nc.scalar.copy(PEPS[0:64, b, :], pss[b][0:64, :]).then_inc(csem, 1)
            scalar.wait_ge(msem, NDR + 2)
            nc.scalar.copy(PEPS[:, 1, :], pss[1][:]).then_inc(csem, 1)

        @block.tensor
        def _(tensor):
            tensor.wait_ge(don, 16)
            for g in range(2):
                tensor.wait_ge(dpe, 32 * (g + 1))
                for c in range(NDR):
                    ins = nc.tensor.matmul(
                        pss[c][0:64, :],
                        ONES[:],
                        XPE[:, 2 * g:2 * g + 2, c * DRW:(c + 1) * DRW],
                        start=(g == 0), stop=(g == 1),
                        perf_mode=mybir.MatmulPerfMode.DoubleRow,
                        skip_group_check=True)
                    if g == 1:
                        ins.then_inc(msem, 1)
                for pl in (2 * g, 2 * g + 1):   # leftover cols: plain fp8 mms
                    for j in range(2):
                        base = NDR * DRW + j * PLW
                        ins = nc.tensor.matmul(
                            pss[j][64:128, 0:PLW],
                            ONES[:, 0, :],
                            XPE[:, pl, base:base + PLW],
                            start=(g == 0 and pl == 0), stop=(g == 1 and pl == 3),
                            skip_group_check=True)
                        if g == 1 and pl == 3:
                            ins.then_inc(msem, 1)

        @block.vector
        def _(vector):
            vector.wait_ge(ddv, 16)
            nc.vector.reduce_sum(
                DPP[:, 0, :], XDV[:].rearrange("k t p -> k p t"),
                axis=mybir.AxisListType.X).then_inc(vsem, 1)
            for b in (2, 4, 6):          # even banks (DR rows only)
                vector.wait_ge(msem, b + 1)
                nc.vector.tensor_copy(PEPS[0:64, b, :], pss[b][0:64, :]).then_inc(csem, 1)
            vector.wait_ge(msem, NDR + 2)
            nc.vector.tensor_copy(PEPS[:, 0, :], pss[0][:]).then_inc(csem, 1)

        @block.gpsimd
        def _(gpsimd):
            gpsimd.wait_ge(dpl, 16)
            nc.gpsimd.tensor_add(PL4[:], XPL[:, 0:4, :], XPL[:, 4:8, :])
            nc.gpsimd.tensor_add(PL4[:, 0:2, :], PL4[:, 0:2, :], PL4[:, 2:4, :])
            nc.gpsimd.tensor_tensor(
                DPP[:, 1, :], PL4[:, 0, :], PL4[:, 1, :],
                mybir.AluOpType.add).then_inc(psem, 1)
    return nc


def _build_apply_nc():
    nc = bass.Bass()
    bf = mybir.dt.bfloat16
    f32 = mybir.dt.float32
    x2 = nc.declare_dram_parameter("x2", [2 * P, L2N], bf, isOutput=False)
    w2 = nc.declare_dram_parameter("w2", [2 * P, 2 * P], bf, isOutput=False)
    y2 = nc.declare_dram_parameter("y2", [2 * P, L2N], bf, isOutput=True)
    NCK = L2N // 512         # 4 column chunks; in/out DMAs move chunk PAIRS

    with bass.ExitStack() as ctx:
        X2 = ctx.enter_context(nc.sbuf_tensor([2 * P, L2N], bf))
        W2 = ctx.enter_context(nc.sbuf_tensor([2 * P, 2 * P], bf))
        OUT2 = ctx.enter_context(nc.sbuf_tensor([2 * P, L2N], bf))
        pss = [ctx.enter_context(nc.psum_tensor(f"ps{c}", [2 * P, 512], f32))
               for c in range(NCK)]
        dsem = ctx.enter_context(nc.semaphore("d"))
        wsem = ctx.enter_context(nc.semaphore("w"))
        msem = ctx.enter_context(nc.semaphore("m"))
        ssem = ctx.enter_context(nc.semaphore("s"))
        vsem = ctx.enter_context(nc.semaphore("v"))
        osem = ctx.enter_context(nc.semaphore("o"))
        block = ctx.enter_context(nc.Block())

        @block.sync
        def _(sync):
            for h in range(2):
                sync.dma_start(X2[:, 1024 * h:1024 * (h + 1)],
                               x2[:, 1024 * h:1024 * (h + 1)]).then_inc(dsem, 16)
            for h in range(2):
                sync.wait_ge(ssem, h + 1)
                sync.wait_ge(vsem, h + 1)
                sync.dma_start(y2[:, 1024 * h:1024 * (h + 1)],
                               OUT2[:, 1024 * h:1024 * (h + 1)]).then_inc(osem, 16)
            sync.wait_ge(osem, 32)

        @block.scalar
        def _(scalar):
            scalar.dma_start(W2[:], w2[:]).then_inc(wsem, 16)
            for c in range(0, NCK, 2):
                scalar.wait_ge(msem, c + 1)
                nc.scalar.copy(OUT2[:, 512 * c:512 * (c + 1)],
                               pss[c][:]).then_inc(ssem, 1)

        @block.tensor
        def _(tensor):
            tensor.wait_ge(wsem, 16)
            for c in range(NCK):
                tensor.wait_ge(dsem, 16 * (c // 2 + 1))
                nc.tensor.matmul(pss[c][:], W2[:],
                                 X2[:, 512 * c:512 * (c + 1)],
                                 start=True, stop=True).then_inc(msem, 1)

        @block.vector
        def _(vector):
            for c in range(1, NCK, 2):
                vector.wait_ge(msem, c + 1)
                nc.vector.tensor_copy(OUT2[:, 512 * c:512 * (c + 1)],
                                      pss[c][:]).then_inc(vsem, 1)
    return nc


def _prep_core_l1(lg):
    """lg [2,100,112,112] f32 -> (xpe [4,PLANE,COLS], xdv, xpl) fp8."""
    t = lg.reshape(BPC, K, 16, ROW, 16, COL).transpose(0, 2, 4, 1, 3, 5)
    t = np.ascontiguousarray(t).reshape(NT, K, P).astype(FP8)   # [512,100,49]
    xpe = np.ascontiguousarray(t[:NT_PE]).reshape(4, PLANE, COLS)
    xdv = np.ascontiguousarray(
        t[NT_PE:NT_PE + NT_DVE].transpose(1, 0, 2))             # [K,24,49]
    xpl = np.ascontiguousarray(
        t[NT_PE + NT_DVE:].transpose(1, 0, 2))                  # [K,8,49]
    return xpe, xdv, xpl


def _prep_core_l2(ft):
    """ft [2,8,112,112] f32 -> x2 [98, 2048] bf16 patch-major."""
    t = ft.reshape(BPC, FEAT_C, 16, ROW, 16, COL)
    t = t.transpose(0, 3, 5, 1, 2, 4)            # [b, r, cc, c, nh, nw]
    return np.ascontiguousarray(t).reshape(2 * P, L2N).astype(BF16)


def _unpack_core_l2(y2):
    """y2 [98, 2048] bf16 -> [2,8,112,112] f32."""
    t = y2.astype(np.float32).reshape(BPC, ROW, COL, FEAT_C, 16, 16)
    t = t.transpose(0, 3, 4, 1, 5, 2)            # [b, c, nh, i, nw, jj]
    return np.ascontiguousarray(t).reshape(BPC, FEAT_C, H, W_)


def kernel(inp, logits, sigma_x, sigma_y, opacity, rho, scale):
    inp = np.asarray(inp)
    logits = np.asarray(logits, dtype=np.float32)
    feat = np.asarray(inp[:, :FEAT_C], dtype=np.float32)

    if "reduce" not in _cache:
        _cache["reduce"] = _build_reduce_nc()
    if "apply" not in _cache:
        _cache["apply"] = _build_apply_nc()

    core_ids = list(range(N_CORES))
    ones8 = np.ones((PLANE, 128), dtype=FP8)
    in_maps1 = []
    for i in core_ids:
        xpe, xdv, xpl = _prep_core_l1(logits[BPC * i:BPC * (i + 1)])
        in_maps1.append({"xpe": xpe, "xdv": xdv, "xpl": xpl, "onesd": ones8})
    res1 = run_bass_kernel_spmd(_cache["reduce"], in_maps1, core_ids)

    lbar_sum = np.zeros((K, P), np.float64)
    for i in core_ids:
        r = res1.results[i]
        pe = np.asarray(r["pe_part"]).astype(np.float64).reshape(-1)   # [4096]
        pe2 = np.asarray(r["pe_part2"]).astype(np.float64).reshape(-1)  # [804]
        lbar_sum += np.concatenate([pe, pe2]).reshape(K, P)
        dvpl = np.asarray(r["dvpl_part"]).astype(np.float64)
        lbar_sum += dvpl[:, 0] + dvpl[:, 1]
    lbar = (lbar_sum / (B * 256)).T              # [49, 100]

    Wm = _compute_W(lbar, np.asarray(sigma_x), np.asarray(sigma_y),
                    np.asarray(opacity), np.asarray(rho))
    W2 = np.zeros((2 * P, 2 * P), np.float32)
    W2[:P, :P] = Wm
    W2[P:, P:] = Wm
    W2 = W2.astype(BF16)

    in_maps2 = [{"x2": _prep_core_l2(feat[BPC * i:BPC * (i + 1)]), "w2": W2}
                for i in core_ids]
    res2 = run_bass_kernel_spmd(_cache["apply"], in_maps2, core_ids)

    out = np.concatenate(
        [_unpack_core_l2(np.asarray(res2.results[i]["y2"])) for i in core_ids],
        axis=0)
    return out.astype(np.float32)

